# revision 2
# baseline (speedup 1.0000x reference)
"""Multi-head attention (B=2, S=2048, D=768, H=12) on 8 trn2 NeuronCores.

Sharding: batch x head-group data/tensor parallel. Core c = b*4+g handles
batch b and heads [3g, 3g+3) (a 192-wide slice of the QKV projections and
the matching 192-row slice of Wo). Each core emits a partial [2048, 768]
output; the host sums the 4 head-group partials per batch and adds bo.

Device layout notes:
- Inputs are transposed on host to [d_model, seq] and cast to fp16 so the
  TensorEngine (which contracts over the partition dim) can consume them
  directly; all matmuls run on fp16 operands with fp32 PSUM accumulation.
- Attention works on transposed scores sT[k, q] so softmax's sum over k
  becomes a matmul reduction: v is augmented with a ones column, so the
  ctx matmul yields both ctx^T and the softmax denominator in one pass.
  exp() needs no max-subtraction: |scores/8| <= ~11 for this problem.
- Normalization multiplies ctx^T by 1/denom broadcast across partitions
  (GPSIMD partition_broadcast), then the output projection runs from
  ctx^T directly.
"""

import numpy as np

D_MODEL = 768
NUM_HEADS = 12
D_K = 64
B = 2
S = 2048
N_CORES = 8
G = 4            # head groups (cores per batch)
GW = D_MODEL // G  # 192 features per group = 3 heads
HPG = 3          # heads per group
DC = D_MODEL // 128  # 6 d_model chunks
QT = 512         # q-tile width
NQT = S // QT    # 4
KC = S // 128    # 16 k chunks
ST = S // 128    # 16 seq tiles

_PROGRAM = None


def _build_program():
    from concourse import bacc, tile
    import concourse.mybir as mybir

    f16 = mybir.dt.float16
    f32 = mybir.dt.float32
    Exp = mybir.ActivationFunctionType.Exp
    mult = mybir.AluOpType.mult

    nc = bacc.Bacc("TRN2", target_bir_lowering=False, debug=False,
                   enable_asserts=False)

    xqT = nc.dram_tensor("xqT", [D_MODEL, S], f16, kind="ExternalInput")
    xkT = nc.dram_tensor("xkT", [D_MODEL, S], f16, kind="ExternalInput")
    xvT = nc.dram_tensor("xvT", [D_MODEL, S], f16, kind="ExternalInput")
    wq = nc.dram_tensor("wq", [D_MODEL, GW], f16, kind="ExternalInput")
    wk = nc.dram_tensor("wk", [D_MODEL, GW], f16, kind="ExternalInput")
    wv = nc.dram_tensor("wv", [D_MODEL, GW], f16, kind="ExternalInput")
    wo = nc.dram_tensor("wo", [GW, D_MODEL], f16, kind="ExternalInput")
    bq = nc.dram_tensor("bq", [GW, 1], f32, kind="ExternalInput")
    bk = nc.dram_tensor("bk", [GW, 1], f32, kind="ExternalInput")
    bv = nc.dram_tensor("bv", [GW, 1], f32, kind="ExternalInput")
    out = nc.dram_tensor("out", [S, D_MODEL], f32, kind="ExternalOutput")

    with tile.TileContext(nc) as tc:
        with tc.tile_pool(name="const", bufs=1) as cp, \
             tc.tile_pool(name="expp", bufs=4) as ep, \
             tc.tile_pool(name="normp", bufs=2) as np_, \
             tc.tile_pool(name="outp", bufs=2) as op, \
             tc.tile_pool(name="ps_s", bufs=4, space="PSUM") as ps_s, \
             tc.tile_pool(name="ps_c", bufs=2, space="PSUM") as ps_c, \
             tc.tile_pool(name="ps_o", bufs=1, space="PSUM") as ps_o:

            # ---- load inputs ----
            xq_sb = []
            xk_sb = []
            xv_sb = []
            for d in range(DC):
                t = cp.tile([128, S], f16, name=f"xq{d}")
                nc.sync.dma_start(out=t[:], in_=xqT[d * 128:(d + 1) * 128, :])
                xq_sb.append(t)
            for d in range(DC):
                t = cp.tile([128, S], f16, name=f"xk{d}")
                nc.sync.dma_start(out=t[:], in_=xkT[d * 128:(d + 1) * 128, :])
                xk_sb.append(t)
            for d in range(DC):
                t = cp.tile([128, S], f16, name=f"xv{d}")
                nc.sync.dma_start(out=t[:], in_=xvT[d * 128:(d + 1) * 128, :])
                xv_sb.append(t)
            wq_sb, wk_sb, wv_sb = [], [], []
            for nm, dram, lst in (("wq", wq, wq_sb), ("wk", wk, wk_sb),
                                  ("wv", wv, wv_sb)):
                for d in range(DC):
                    t = cp.tile([128, GW], f16, name=f"{nm}{d}")
                    nc.sync.dma_start(out=t[:], in_=dram[d * 128:(d + 1) * 128, :])
                    lst.append(t)
            wo_a = cp.tile([128, D_MODEL], f16, name="wo_a")
            nc.sync.dma_start(out=wo_a[:], in_=wo[0:128, :])
            wo_b = cp.tile([64, D_MODEL], f16, name="wo_b")
            nc.sync.dma_start(out=wo_b[:], in_=wo[128:GW, :])
            bq_a = cp.tile([128, 1], f32, name="bq_a")
            nc.sync.dma_start(out=bq_a[:], in_=bq[0:128, :])
            bq_b = cp.tile([64, 1], f32, name="bq_b")
            nc.sync.dma_start(out=bq_b[:], in_=bq[128:GW, :])
            bk_a = cp.tile([128, 1], f32, name="bk_a")
            nc.sync.dma_start(out=bk_a[:], in_=bk[0:128, :])
            bk_b = cp.tile([64, 1], f32, name="bk_b")
            nc.sync.dma_start(out=bk_b[:], in_=bk[128:GW, :])
            bv_h = []
            for h in range(HPG):
                t = cp.tile([64, 1], f32, name=f"bv{h}")
                nc.sync.dma_start(out=t[:], in_=bv[h * 64:(h + 1) * 64, :])
                bv_h.append(t)

            # ---- q/k projections: qT = Wq.T @ x.T, [192, S] as [128]+[64] ----
            qT_a = cp.tile([128, S], f16, name="qT_a")
            qT_b = cp.tile([64, S], f16, name="qT_b")
            kT_a = cp.tile([128, S], f16, name="kT_a")
            kT_b = cp.tile([64, S], f16, name="kT_b")
            for x_sb, w_sb, b_a, b_b, dst_a, dst_b in (
                    (xq_sb, wq_sb, bq_a, bq_b, qT_a, qT_b),
                    (xk_sb, wk_sb, bk_a, bk_b, kT_a, kT_b)):
                for j in range(NQT):
                    cs = slice(j * QT, (j + 1) * QT)
                    pj = ps_c.tile([128, QT], f32, name="pj", tag="c")
                    for d in range(DC):
                        nc.tensor.matmul(pj[:], lhsT=w_sb[d][:, 0:128],
                                         rhs=x_sb[d][:, cs],
                                         start=(d == 0), stop=(d == DC - 1))
                    nc.vector.tensor_scalar_add(dst_a[:, cs], pj[:], b_a[:])
                    pj2 = ps_c.tile([64, QT], f32, name="pj", tag="c")
                    for d in range(DC):
                        nc.tensor.matmul(pj2[:], lhsT=w_sb[d][:, 128:GW],
                                         rhs=x_sb[d][:, cs],
                                         start=(d == 0), stop=(d == DC - 1))
                    nc.vector.tensor_scalar_add(dst_b[:, cs], pj2[:], b_b[:])

            # ---- v projection (natural layout) + ones column per head ----
            v_sb = []
            for st in range(ST):
                rs = slice(st * 128, (st + 1) * 128)
                pv = ps_c.tile([128, GW], f32, name="pj", tag="c")
                for d in range(DC):
                    nc.tensor.matmul(pv[:], lhsT=xv_sb[d][:, rs],
                                     rhs=wv_sb[d][:],
                                     start=(d == 0), stop=(d == DC - 1))
                vt = cp.tile([128, HPG, D_K + 1], f16, name=f"vsb{st}")
                nc.vector.tensor_copy(out=vt[:, :, 0:D_K],
                                      in_=pv.rearrange("p (h w) -> p h w", h=HPG))
                nc.vector.memset(vt[:, :, D_K:D_K + 1], 1.0)
                v_sb.append(vt)

            # ---- attention (transposed scores) + output projection ----
            ctxT_a = cp.tile([128, S], f16, name="ctxT_a")
            ctxT_b = cp.tile([64, S], f16, name="ctxT_b")

            def head_slices(h):
                if h == 0:
                    return kT_a[0:64], qT_a[0:64], ctxT_a[0:64]
                if h == 1:
                    return kT_a[64:128], qT_a[64:128], ctxT_a[64:128]
                return kT_b[0:64], qT_b[0:64], ctxT_b[0:64]

            for qt in range(NQT):
                qs = slice(qt * QT, (qt + 1) * QT)
                for hp in ((0, 1), (2,)):
                    Cs = {}
                    for h in hp:
                        Cs[h] = ps_c.tile([D_K + 1, QT], f32, name="C", tag="c")
                    for kc in range(KC):
                        ks = slice(kc * 128, (kc + 1) * 128)
                        Ss = {}
                        for h in hp:
                            kT_h, qT_h, _ = head_slices(h)
                            Sh = ps_s.tile([128, QT], f32, name="S", tag="s")
                            nc.tensor.matmul(Sh[:], lhsT=kT_h[:, ks],
                                             rhs=qT_h[:, qs])
                            Ss[h] = Sh
                        es = {}
                        for h in hp:
                            e = ep.tile([128, QT], f16, name="expT")
                            nc.scalar.activation(e[:], Ss[h][:], Exp, scale=0.125)
                            es[h] = e
                        for h in hp:
                            nc.tensor.matmul(Cs[h][:], lhsT=v_sb[kc][:, h, :],
                                             rhs=es[h][:],
                                             start=(kc == 0), stop=(kc == KC - 1))
                    for h in hp:
                        _, _, ctx_dst = head_slices(h)
                        r = np_.tile([1, QT], f16, name="r")
                        with nc.allow_low_precision(reason="softmax recip in f16"):
                            nc.vector.reciprocal(r[:], Cs[h][D_K:D_K + 1, :])
                        bc = np_.tile([128, QT], f16, name="bc")
                        nc.gpsimd.partition_broadcast(bc[:], r[:])
                        base = 64 if h == 1 else 0
                        nc.vector.tensor_tensor(out=ctx_dst[:, qs],
                                                in0=Cs[h][0:D_K, :],
                                                in1=bc[base:base + D_K, :],
                                                op=mult)
                        nc.vector.tensor_scalar_add(ctx_dst[:, qs],
                                                    ctx_dst[:, qs],
                                                    bv_h[h][:])
                # output projection for these 512 query rows (4 seq tiles)
                for st in range(QT // 128):
                    r0 = qt * QT + st * 128
                    ws = slice(r0, r0 + 128)
                    po = ps_o.tile([128, D_MODEL], f32, name="po", tag="po")
                    for n, ns in enumerate((slice(0, 512), slice(512, 768))):
                        nc.tensor.matmul(po[:, ns], lhsT=ctxT_a[:, ws],
                                         rhs=wo_a[:, ns], start=True, stop=False)
                        nc.tensor.matmul(po[:, ns], lhsT=ctxT_b[:, ws],
                                         rhs=wo_b[:, ns], start=False, stop=True)
                    osb = op.tile([128, D_MODEL], f32, name="osb")
                    nc.vector.tensor_copy(out=osb[:], in_=po[:])
                    nc.sync.dma_start(out=out[ws, :], in_=osb[:])

    nc.compile()
    return nc


def _get_program():
    global _PROGRAM
    if _PROGRAM is None:
        _PROGRAM = _build_program()
    return _PROGRAM


def make_in_maps(query, key, value, Wq, bq, Wk, bk, Wv, bv, Wo, bo):
    """Build the 8 per-core input maps (host-side shard + transpose + cast)."""
    q32 = np.asarray(query, np.float32)
    k32 = np.asarray(key, np.float32)
    v32 = np.asarray(value, np.float32)
    xT = {}
    for b in range(B):
        xT[b] = (np.ascontiguousarray(q32[b].T).astype(np.float16),
                 np.ascontiguousarray(k32[b].T).astype(np.float16),
                 np.ascontiguousarray(v32[b].T).astype(np.float16))
    Wq = np.asarray(Wq, np.float32)
    Wk = np.asarray(Wk, np.float32)
    Wv = np.asarray(Wv, np.float32)
    Wo = np.asarray(Wo, np.float32)
    in_maps = []
    for c in range(N_CORES):
        b, g = divmod(c, G)
        fs = slice(g * GW, (g + 1) * GW)
        xq, xk, xv = xT[b]
        in_maps.append({
            "xqT": xq, "xkT": xk, "xvT": xv,
            "wq": np.ascontiguousarray(Wq[:, fs]).astype(np.float16),
            "wk": np.ascontiguousarray(Wk[:, fs]).astype(np.float16),
            "wv": np.ascontiguousarray(Wv[:, fs]).astype(np.float16),
            "wo": np.ascontiguousarray(Wo[fs, :]).astype(np.float16),
            "bq": np.asarray(bq, np.float32)[fs].reshape(GW, 1),
            "bk": np.asarray(bk, np.float32)[fs].reshape(GW, 1),
            "bv": np.asarray(bv, np.float32)[fs].reshape(GW, 1),
        })
    return in_maps


def combine_outputs(results, bo):
    """Sum the per-core partial outputs into the full [B, S, D] output."""
    bo = np.asarray(bo, np.float32)
    out = np.zeros((B, S, D_MODEL), np.float32)
    for c in range(N_CORES):
        b = c // G
        out[b] += np.asarray(results[c]["out"], np.float32)
    out += bo[None, None, :]
    return out


def kernel(**inputs):
    from concourse.bass_utils import run_bass_kernel_spmd

    nc = _get_program()
    in_maps = make_in_maps(**inputs)
    res = run_bass_kernel_spmd(nc, in_maps, list(range(N_CORES)))
    return combine_outputs(res.results, inputs["bo"])


# revision 14
# speedup vs baseline: 1.1731x; 1.1731x over previous
"""Multi-head attention (B=2, S=2048, D=768, H=12) on 8 trn2 NeuronCores.

Sharding: batch x head-group data/tensor parallel. Core c = b*4+g handles
batch b and heads [3g, 3g+3) (a 192-wide slice of the QKV projections and
the matching 192-row slice of Wo). Each core emits a partial [2048, 768]
output; the host sums the 4 head-group partials per batch and adds bo.

Device layout notes:
- Inputs are transposed on host to [d_model, seq] and cast to fp16 so the
  TensorEngine (which contracts over the partition dim) can consume them
  directly; all matmuls run on fp16 operands with fp32 PSUM accumulation.
- Attention works on transposed scores sT[k, q] so softmax's sum over k
  becomes a matmul reduction: v is augmented with a ones column, so the
  ctx matmul yields both ctx^T and the softmax denominator in one pass.
  exp() needs no max-subtraction: |scores/8| <= ~11 for this problem.
- Normalization multiplies ctx^T by 1/denom broadcast across partitions
  (GPSIMD partition_broadcast), then the output projection runs from
  ctx^T directly.
"""

import numpy as np

D_MODEL = 768
NUM_HEADS = 12
D_K = 64
B = 2
S = 2048
N_CORES = 8
G = 4            # head groups (cores per batch)
GW = D_MODEL // G  # 192 features per group = 3 heads
HPG = 3          # heads per group
DC = D_MODEL // 128  # 6 d_model chunks
QT = 512         # q-tile width
NQT = S // QT    # 4
KC = S // 128    # 16 k chunks
ST = S // 128    # 16 seq tiles

_PROGRAM = None


def _build_program():
    from concourse import bacc, tile
    import concourse.mybir as mybir

    f16 = mybir.dt.float16
    f32 = mybir.dt.float32
    Exp = mybir.ActivationFunctionType.Exp
    mult = mybir.AluOpType.mult

    nc = bacc.Bacc("TRN2", target_bir_lowering=False, debug=False,
                   enable_asserts=False)

    xqT = nc.dram_tensor("xqT", [D_MODEL, S], f16, kind="ExternalInput")
    xkT = nc.dram_tensor("xkT", [D_MODEL, S], f16, kind="ExternalInput")
    xvT = nc.dram_tensor("xvT", [D_MODEL, S], f16, kind="ExternalInput")
    wq = nc.dram_tensor("wq", [D_MODEL, GW], f16, kind="ExternalInput")
    wk = nc.dram_tensor("wk", [D_MODEL, GW], f16, kind="ExternalInput")
    wv = nc.dram_tensor("wv", [D_MODEL, GW], f16, kind="ExternalInput")
    wo = nc.dram_tensor("wo", [GW, D_MODEL], f16, kind="ExternalInput")
    bq = nc.dram_tensor("bq", [GW, 1], f32, kind="ExternalInput")
    bk = nc.dram_tensor("bk", [GW, 1], f32, kind="ExternalInput")
    bv = nc.dram_tensor("bv", [GW, 1], f32, kind="ExternalInput")
    out = nc.dram_tensor("out", [S, D_MODEL], f32, kind="ExternalOutput")

    with tile.TileContext(nc) as tc:
        with tc.tile_pool(name="const", bufs=1) as cp, \
             tc.tile_pool(name="expp", bufs=4) as ep, \
             tc.tile_pool(name="normp", bufs=2) as np_, \
             tc.tile_pool(name="outp", bufs=2) as op, \
             tc.tile_pool(name="ps_s", bufs=2, space="PSUM") as ps_s, \
             tc.tile_pool(name="ps_c", bufs=2, space="PSUM") as ps_c, \
             tc.tile_pool(name="ps_o", bufs=1, space="PSUM") as ps_o:

            # ---- load inputs (k first, then v, then q: attention needs
            # full kT and v before it can start, q only per-tile) ----
            xq_sb = []
            xk_sb = []
            xv_sb = []
            for d in range(DC):
                t = cp.tile([128, S], f16, name=f"xk{d}")
                nc.sync.dma_start(out=t[:], in_=xkT[d * 128:(d + 1) * 128, :])
                xk_sb.append(t)
            for d in range(DC):
                t = cp.tile([128, S], f16, name=f"xv{d}")
                nc.sync.dma_start(out=t[:], in_=xvT[d * 128:(d + 1) * 128, :])
                xv_sb.append(t)
            for d in range(DC):
                t = cp.tile([128, S], f16, name=f"xq{d}")
                nc.sync.dma_start(out=t[:], in_=xqT[d * 128:(d + 1) * 128, :])
                xq_sb.append(t)
            wq_sb, wk_sb, wv_sb = [], [], []
            for nm, dram, lst in (("wq", wq, wq_sb), ("wk", wk, wk_sb),
                                  ("wv", wv, wv_sb)):
                for d in range(DC):
                    t = cp.tile([128, GW], f16, name=f"{nm}{d}")
                    nc.sync.dma_start(out=t[:], in_=dram[d * 128:(d + 1) * 128, :])
                    lst.append(t)
            wo_a = cp.tile([128, D_MODEL], f16, name="wo_a")
            nc.sync.dma_start(out=wo_a[:], in_=wo[0:128, :])
            wo_b = cp.tile([64, D_MODEL], f16, name="wo_b")
            nc.sync.dma_start(out=wo_b[:], in_=wo[128:GW, :])
            bq_a = cp.tile([128, 1], f32, name="bq_a")
            nc.sync.dma_start(out=bq_a[:], in_=bq[0:128, :])
            bq_b = cp.tile([64, 1], f32, name="bq_b")
            nc.sync.dma_start(out=bq_b[:], in_=bq[128:GW, :])
            bk_a = cp.tile([128, 1], f32, name="bk_a")
            nc.sync.dma_start(out=bk_a[:], in_=bk[0:128, :])
            bk_b = cp.tile([64, 1], f32, name="bk_b")
            nc.sync.dma_start(out=bk_b[:], in_=bk[128:GW, :])
            bv_h = []
            for h in range(HPG):
                t = cp.tile([64, 1], f32, name=f"bv{h}")
                nc.sync.dma_start(out=t[:], in_=bv[h * 64:(h + 1) * 64, :])
                bv_h.append(t)

            # ---- projections. Order: kT, v, qT (attention dependency order)
            qT_a = cp.tile([128, S], f16, name="qT_a")
            qT_b = cp.tile([128, S], f16, name="qT_b")
            kT_a = cp.tile([128, S], f16, name="kT_a")
            kT_b = cp.tile([128, S], f16, name="kT_b")

            def qk_proj(x_sb, w_sb, b_a, b_b, dst_a, dst_b):
                for j in range(NQT):
                    cs = slice(j * QT, (j + 1) * QT)
                    pj = ps_c.tile([128, QT], f32, name="pj", tag="c")
                    for d in range(DC):
                        nc.tensor.matmul(pj[:], lhsT=w_sb[d][:, 0:128],
                                         rhs=x_sb[d][:, cs],
                                         start=(d == 0), stop=(d == DC - 1))
                    nc.vector.tensor_scalar_add(dst_a[:, cs], pj[:], b_a[:])
                    pj2 = ps_c.tile([64, QT], f32, name="pj", tag="c")
                    for d in range(DC):
                        nc.tensor.matmul(pj2[:], lhsT=w_sb[d][:, 128:GW],
                                         rhs=x_sb[d][:, cs],
                                         start=(d == 0), stop=(d == DC - 1))
                    nc.vector.tensor_scalar_add(dst_b[0:64, cs], pj2[:], b_b[:])
                # mirror the 64-row b-half into partitions 64-127 so head-2
                # score matmuls can alternate PE row groups (pairing)
                nc.sync.dma_start(out=dst_b[64:128, :], in_=dst_b[0:64, :])

            qk_proj(xk_sb, wk_sb, bk_a, bk_b, kT_a, kT_b)

            # v projection (natural layout) + ones column per head
            v_sb = []
            for st in range(ST):
                rs = slice(st * 128, (st + 1) * 128)
                pv = ps_c.tile([128, GW], f32, name="pj", tag="c")
                for d in range(DC):
                    nc.tensor.matmul(pv[:], lhsT=xv_sb[d][:, rs],
                                     rhs=wv_sb[d][:],
                                     start=(d == 0), stop=(d == DC - 1))
                vt = cp.tile([128, HPG, D_K + 1], f16, name=f"vsb{st}")
                nc.vector.tensor_copy(out=vt[:, :, 0:D_K],
                                      in_=pv.rearrange("p (h w) -> p h w", h=HPG))
                nc.vector.memset(vt[:, :, D_K:D_K + 1], 1.0)
                v_sb.append(vt)

            qk_proj(xq_sb, wq_sb, bq_a, bq_b, qT_a, qT_b)

            # ---- attention (transposed scores) + output projection ----
            ctxT_a = cp.tile([128, S], f16, name="ctxT_a")
            ctxT_b = cp.tile([64, S], f16, name="ctxT_b")

            def head_slices(h):
                if h == 0:
                    return kT_a[0:64], qT_a[0:64], ctxT_a[0:64]
                if h == 1:
                    return kT_a[64:128], qT_a[64:128], ctxT_a[64:128]
                return kT_b[0:64], qT_b[0:64], ctxT_b[0:64]

            def normalize(C, h, qs):
                # ctxT = C[0:64] * (1/denom) + bv, denom = C[64] (psum)
                _, _, ctx_dst = head_slices(h)
                r = np_.tile([1, QT], f16, name="r")
                with nc.allow_low_precision(reason="softmax recip in f16"):
                    nc.vector.reciprocal(r[:], C[D_K:D_K + 1, :])
                bc = np_.tile([128, QT], f16, name="bc")
                nc.gpsimd.partition_broadcast(bc[:], r[:])
                base = 64 if h == 1 else 0
                nc.vector.tensor_tensor(out=ctx_dst[:, qs],
                                        in0=C[0:D_K, :],
                                        in1=bc[base:base + D_K, :],
                                        op=mult)
                nc.vector.tensor_scalar_add(ctx_dst[:, qs], ctx_dst[:, qs],
                                            bv_h[h][:])

            for qt in range(NQT):
                qs = slice(qt * QT, (qt + 1) * QT)
                # heads 0+1 interleaved: scores for both go into one
                # [128, 1024] PSUM tile so exp runs as a single wide op,
                # and the two matmuls (row groups 0-63 / 64-127) overlap.
                Cs = {}
                for h in (0, 1):
                    Cs[h] = ps_c.tile([D_K + 1, QT], f32, name="C", tag="c")
                for kc in range(KC):
                    ks = slice(kc * 128, (kc + 1) * 128)
                    S2 = ps_s.tile([128, 2 * QT], f32, name="S", tag="s")
                    for h in (0, 1):
                        kT_h, qT_h, _ = head_slices(h)
                        nc.tensor.matmul(S2[:, h * QT:(h + 1) * QT],
                                         lhsT=kT_h[:, ks], rhs=qT_h[:, qs])
                    e2 = ep.tile([128, 2 * QT], f16, name="expT")
                    nc.scalar.activation(e2[:], S2[:], Exp, scale=0.125)
                    for h in (0, 1):
                        nc.tensor.matmul(Cs[h][:], lhsT=v_sb[kc][:, h, :],
                                         rhs=e2[:, h * QT:(h + 1) * QT],
                                         start=(kc == 0), stop=(kc == KC - 1))
                for h in (0, 1):
                    normalize(Cs[h], h, qs)
                # head 2: one [128, 1024] scores tile covers two k-chunks;
                # alternate PE row groups via the mirrored b-half
                C2 = ps_c.tile([D_K + 1, QT], f32, name="C", tag="c")
                for kc2 in range(KC // 2):
                    S2 = ps_s.tile([128, 2 * QT], f32, name="S", tag="s")
                    for i in (0, 1):
                        kc = 2 * kc2 + i
                        rg = slice(64 * i, 64 * i + 64)
                        nc.tensor.matmul(S2[:, i * QT:(i + 1) * QT],
                                         lhsT=kT_b[rg, kc * 128:(kc + 1) * 128],
                                         rhs=qT_b[rg, qs])
                    e2 = ep.tile([128, 2 * QT], f16, name="expT")
                    nc.scalar.activation(e2[:], S2[:], Exp, scale=0.125)
                    for i in (0, 1):
                        kc = 2 * kc2 + i
                        nc.tensor.matmul(C2[:], lhsT=v_sb[kc][:, 2, :],
                                         rhs=e2[:, i * QT:(i + 1) * QT],
                                         start=(kc == 0), stop=(kc == KC - 1))
                normalize(C2, 2, qs)
                # output projection for these 512 query rows (4 seq tiles)
                for st in range(QT // 128):
                    r0 = qt * QT + st * 128
                    ws = slice(r0, r0 + 128)
                    po = ps_o.tile([128, D_MODEL], f32, name="po", tag="po")
                    for n, ns in enumerate((slice(0, 512), slice(512, 768))):
                        nc.tensor.matmul(po[:, ns], lhsT=ctxT_a[:, ws],
                                         rhs=wo_a[:, ns], start=True, stop=False)
                        nc.tensor.matmul(po[:, ns], lhsT=ctxT_b[:, ws],
                                         rhs=wo_b[:, ns], start=False, stop=True)
                    osb = op.tile([128, D_MODEL], f32, name="osb")
                    nc.vector.tensor_copy(out=osb[:], in_=po[:])
                    nc.sync.dma_start(out=out[ws, :], in_=osb[:])

    nc.compile()
    return nc


def _get_program():
    global _PROGRAM
    if _PROGRAM is None:
        _PROGRAM = _build_program()
    return _PROGRAM


def make_in_maps(query, key, value, Wq, bq, Wk, bk, Wv, bv, Wo, bo):
    """Build the 8 per-core input maps (host-side shard + transpose + cast)."""
    q32 = np.asarray(query, np.float32)
    k32 = np.asarray(key, np.float32)
    v32 = np.asarray(value, np.float32)
    xT = {}
    for b in range(B):
        xT[b] = (np.ascontiguousarray(q32[b].T).astype(np.float16),
                 np.ascontiguousarray(k32[b].T).astype(np.float16),
                 np.ascontiguousarray(v32[b].T).astype(np.float16))
    Wq = np.asarray(Wq, np.float32)
    Wk = np.asarray(Wk, np.float32)
    Wv = np.asarray(Wv, np.float32)
    Wo = np.asarray(Wo, np.float32)
    in_maps = []
    for c in range(N_CORES):
        b, g = divmod(c, G)
        fs = slice(g * GW, (g + 1) * GW)
        xq, xk, xv = xT[b]
        in_maps.append({
            "xqT": xq, "xkT": xk, "xvT": xv,
            "wq": np.ascontiguousarray(Wq[:, fs]).astype(np.float16),
            "wk": np.ascontiguousarray(Wk[:, fs]).astype(np.float16),
            "wv": np.ascontiguousarray(Wv[:, fs]).astype(np.float16),
            "wo": np.ascontiguousarray(Wo[fs, :]).astype(np.float16),
            "bq": np.asarray(bq, np.float32)[fs].reshape(GW, 1),
            "bk": np.asarray(bk, np.float32)[fs].reshape(GW, 1),
            "bv": np.asarray(bv, np.float32)[fs].reshape(GW, 1),
        })
    return in_maps


def combine_outputs(results, bo):
    """Sum the per-core partial outputs into the full [B, S, D] output."""
    bo = np.asarray(bo, np.float32)
    out = np.zeros((B, S, D_MODEL), np.float32)
    for c in range(N_CORES):
        b = c // G
        out[b] += np.asarray(results[c]["out"], np.float32)
    out += bo[None, None, :]
    return out


def kernel(**inputs):
    from concourse.bass_utils import run_bass_kernel_spmd

    nc = _get_program()
    in_maps = make_in_maps(**inputs)
    res = run_bass_kernel_spmd(nc, in_maps, list(range(N_CORES)))
    return combine_outputs(res.results, inputs["bo"])


# revision 16
# speedup vs baseline: 1.2093x; 1.0309x over previous
"""Multi-head attention (B=2, S=2048, D=768, H=12) on 8 trn2 NeuronCores.

Sharding: batch x head-group data/tensor parallel. Core c = b*4+g handles
batch b and heads [3g, 3g+3) (a 192-wide slice of the QKV projections and
the matching 192-row slice of Wo). Each core emits a partial [2048, 768]
output; the host sums the 4 head-group partials per batch and adds bo.

Device layout notes:
- Inputs are transposed on host to [d_model, seq] and cast to fp16 so the
  TensorEngine (which contracts over the partition dim) can consume them
  directly; all matmuls run on fp16 operands with fp32 PSUM accumulation.
- Attention works on transposed scores sT[k, q] so softmax's sum over k
  becomes a matmul reduction: v is augmented with a ones column, so the
  ctx matmul yields both ctx^T and the softmax denominator in one pass.
  exp() needs no max-subtraction: |scores/8| <= ~11 for this problem.
- Normalization multiplies ctx^T by 1/denom broadcast across partitions
  (GPSIMD partition_broadcast), then the output projection runs from
  ctx^T directly.
"""

import numpy as np

D_MODEL = 768
NUM_HEADS = 12
D_K = 64
B = 2
S = 2048
N_CORES = 8
G = 4            # head groups (cores per batch)
GW = D_MODEL // G  # 192 features per group = 3 heads
HPG = 3          # heads per group
DC = D_MODEL // 128  # 6 d_model chunks
QT = 512         # q-tile width
NQT = S // QT    # 4
KC = S // 128    # 16 k chunks
ST = S // 128    # 16 seq tiles

_PROGRAM = None


def _build_program():
    from concourse import bacc, tile
    import concourse.mybir as mybir

    f16 = mybir.dt.float16
    f32 = mybir.dt.float32
    Exp = mybir.ActivationFunctionType.Exp
    mult = mybir.AluOpType.mult

    nc = bacc.Bacc("TRN2", target_bir_lowering=False, debug=False,
                   enable_asserts=False)

    xqT = nc.dram_tensor("xqT", [D_MODEL, S], f16, kind="ExternalInput")
    xkT = nc.dram_tensor("xkT", [D_MODEL, S], f16, kind="ExternalInput")
    xvT = nc.dram_tensor("xvT", [D_MODEL, S], f16, kind="ExternalInput")
    wq = nc.dram_tensor("wq", [D_MODEL, GW], f16, kind="ExternalInput")
    wk = nc.dram_tensor("wk", [D_MODEL, GW], f16, kind="ExternalInput")
    wv = nc.dram_tensor("wv", [D_MODEL, GW], f16, kind="ExternalInput")
    wo = nc.dram_tensor("wo", [GW, D_MODEL], f16, kind="ExternalInput")
    bq = nc.dram_tensor("bq", [GW, 1], f32, kind="ExternalInput")
    bk = nc.dram_tensor("bk", [GW, 1], f32, kind="ExternalInput")
    bv = nc.dram_tensor("bv", [GW, 1], f32, kind="ExternalInput")
    out = nc.dram_tensor("out", [S, D_MODEL], f32, kind="ExternalOutput")

    with tile.TileContext(nc) as tc:
        with tc.tile_pool(name="const", bufs=1) as cp, \
             tc.tile_pool(name="expp", bufs=4) as ep, \
             tc.tile_pool(name="normp", bufs=2) as np_, \
             tc.tile_pool(name="outp", bufs=2) as op, \
             tc.tile_pool(name="ps_s", bufs=2, space="PSUM") as ps_s, \
             tc.tile_pool(name="ps_c", bufs=2, space="PSUM") as ps_c, \
             tc.tile_pool(name="ps_o", bufs=1, space="PSUM") as ps_o:

            # ---- load inputs (k first, then v, then q: attention needs
            # full kT and v before it can start, q only per-tile) ----
            xq_sb = []
            xk_sb = []
            xv_sb = []
            for d in range(DC):
                t = cp.tile([128, S], f16, name=f"xk{d}")
                nc.sync.dma_start(out=t[:], in_=xkT[d * 128:(d + 1) * 128, :])
                xk_sb.append(t)
            for d in range(DC):
                t = cp.tile([128, S], f16, name=f"xv{d}")
                nc.sync.dma_start(out=t[:], in_=xvT[d * 128:(d + 1) * 128, :])
                xv_sb.append(t)
            for d in range(DC):
                t = cp.tile([128, S], f16, name=f"xq{d}")
                nc.sync.dma_start(out=t[:], in_=xqT[d * 128:(d + 1) * 128, :])
                xq_sb.append(t)
            wq_sb, wk_sb, wv_sb = [], [], []
            for nm, dram, lst in (("wq", wq, wq_sb), ("wk", wk, wk_sb),
                                  ("wv", wv, wv_sb)):
                for d in range(DC):
                    t = cp.tile([128, GW], f16, name=f"{nm}{d}")
                    nc.sync.dma_start(out=t[:], in_=dram[d * 128:(d + 1) * 128, :])
                    lst.append(t)
            wo_a = cp.tile([128, D_MODEL], f16, name="wo_a")
            nc.sync.dma_start(out=wo_a[:], in_=wo[0:128, :])
            wo_b = cp.tile([64, D_MODEL], f16, name="wo_b")
            nc.sync.dma_start(out=wo_b[:], in_=wo[128:GW, :])
            bq_a = cp.tile([128, 1], f32, name="bq_a")
            nc.sync.dma_start(out=bq_a[:], in_=bq[0:128, :])
            bq_b = cp.tile([64, 1], f32, name="bq_b")
            nc.sync.dma_start(out=bq_b[:], in_=bq[128:GW, :])
            bk_a = cp.tile([128, 1], f32, name="bk_a")
            nc.sync.dma_start(out=bk_a[:], in_=bk[0:128, :])
            bk_b = cp.tile([64, 1], f32, name="bk_b")
            nc.sync.dma_start(out=bk_b[:], in_=bk[128:GW, :])
            bv_h = []
            for h in range(HPG):
                t = cp.tile([64, 1], f32, name=f"bv{h}")
                nc.sync.dma_start(out=t[:], in_=bv[h * 64:(h + 1) * 64, :])
                bv_h.append(t)

            # ---- projections. Order: kT, v, qT (attention dependency order)
            qT_a = cp.tile([128, S], f16, name="qT_a")
            qT_b = cp.tile([128, S], f16, name="qT_b")
            kT_a = cp.tile([128, S], f16, name="kT_a")
            kT_b = cp.tile([128, S], f16, name="kT_b")

            def qk_proj(x_sb, w_sb, b_a, b_b, dst_a, dst_b):
                for j in range(NQT):
                    cs = slice(j * QT, (j + 1) * QT)
                    pj = ps_c.tile([128, QT], f32, name="pj", tag="c")
                    for d in range(DC):
                        nc.tensor.matmul(pj[:], lhsT=w_sb[d][:, 0:128],
                                         rhs=x_sb[d][:, cs],
                                         start=(d == 0), stop=(d == DC - 1))
                    nc.vector.tensor_scalar_add(dst_a[:, cs], pj[:], b_a[:])
                    pj2 = ps_c.tile([64, QT], f32, name="pj", tag="c")
                    for d in range(DC):
                        nc.tensor.matmul(pj2[:], lhsT=w_sb[d][:, 128:GW],
                                         rhs=x_sb[d][:, cs],
                                         start=(d == 0), stop=(d == DC - 1))
                    nc.vector.tensor_scalar_add(dst_b[0:64, cs], pj2[:], b_b[:])
                # mirror the 64-row b-half into partitions 64-127 so head-2
                # score matmuls can alternate PE row groups (pairing)
                nc.sync.dma_start(out=dst_b[64:128, :], in_=dst_b[0:64, :])

            qk_proj(xk_sb, wk_sb, bk_a, bk_b, kT_a, kT_b)

            # v projection (natural layout) + ones column per head
            v_sb = []
            for st in range(ST):
                rs = slice(st * 128, (st + 1) * 128)
                pv = ps_c.tile([128, GW], f32, name="pj", tag="c")
                for d in range(DC):
                    nc.tensor.matmul(pv[:], lhsT=xv_sb[d][:, rs],
                                     rhs=wv_sb[d][:],
                                     start=(d == 0), stop=(d == DC - 1))
                vt = cp.tile([128, HPG, D_K + 1], f16, name=f"vsb{st}")
                nc.vector.tensor_copy(out=vt[:, :, 0:D_K],
                                      in_=pv.rearrange("p (h w) -> p h w", h=HPG))
                nc.vector.memset(vt[:, :, D_K:D_K + 1], 1.0)
                v_sb.append(vt)

            qk_proj(xq_sb, wq_sb, bq_a, bq_b, qT_a, qT_b)

            # ---- attention (transposed scores) + output projection ----
            ctxT_a = cp.tile([128, S], f16, name="ctxT_a")
            ctxT_b = cp.tile([64, S], f16, name="ctxT_b")

            def head_slices(h):
                if h == 0:
                    return kT_a[0:64], qT_a[0:64], ctxT_a[0:64]
                if h == 1:
                    return kT_a[64:128], qT_a[64:128], ctxT_a[64:128]
                return kT_b[0:64], qT_b[0:64], ctxT_b[0:64]

            def normalize(C, h, qs):
                # ctxT = C[0:64] * (1/denom) + bv.  reciprocal_approx_fast
                # must read SBUF (garbage from PSUM on HW), so stage the
                # denominator row through SBUF first.
                _, _, ctx_dst = head_slices(h)
                den = np_.tile([1, QT], f32, name="den")
                nc.vector.tensor_copy(out=den[:], in_=C[D_K:D_K + 1, :])
                r = np_.tile([1, QT], f32, name="r")
                nc.vector.reciprocal_approx_fast(out=r[:], in_=den[:])
                bc = np_.tile([128, QT], f32, name="bc")
                nc.gpsimd.partition_broadcast(bc[:], r[:])
                base = 64 if h == 1 else 0
                nc.vector.tensor_tensor(out=ctx_dst[:, qs],
                                        in0=C[0:D_K, :],
                                        in1=bc[base:base + D_K, :],
                                        op=mult)
                nc.vector.tensor_scalar_add(ctx_dst[:, qs], ctx_dst[:, qs],
                                            bv_h[h][:])

            for qt in range(NQT):
                qs = slice(qt * QT, (qt + 1) * QT)
                # heads 0+1 interleaved: scores for both go into one
                # [128, 1024] PSUM tile so exp runs as a single wide op,
                # and the two matmuls (row groups 0-63 / 64-127) overlap.
                Cs = {}
                for h in (0, 1):
                    Cs[h] = ps_c.tile([D_K + 1, QT], f32, name="C", tag="c")
                for kc in range(KC):
                    ks = slice(kc * 128, (kc + 1) * 128)
                    S2 = ps_s.tile([128, 2 * QT], f32, name="S", tag="s")
                    for h in (0, 1):
                        kT_h, qT_h, _ = head_slices(h)
                        nc.tensor.matmul(S2[:, h * QT:(h + 1) * QT],
                                         lhsT=kT_h[:, ks], rhs=qT_h[:, qs])
                    e2 = ep.tile([128, 2 * QT], f16, name="expT")
                    nc.scalar.activation(e2[:], S2[:], Exp, scale=0.125)
                    for h in (0, 1):
                        nc.tensor.matmul(Cs[h][:], lhsT=v_sb[kc][:, h, :],
                                         rhs=e2[:, h * QT:(h + 1) * QT],
                                         start=(kc == 0), stop=(kc == KC - 1))
                for h in (0, 1):
                    normalize(Cs[h], h, qs)
                # head 2: one [128, 1024] scores tile covers two k-chunks;
                # alternate PE row groups via the mirrored b-half
                C2 = ps_c.tile([D_K + 1, QT], f32, name="C", tag="c")
                for kc2 in range(KC // 2):
                    S2 = ps_s.tile([128, 2 * QT], f32, name="S", tag="s")
                    for i in (0, 1):
                        kc = 2 * kc2 + i
                        rg = slice(64 * i, 64 * i + 64)
                        nc.tensor.matmul(S2[:, i * QT:(i + 1) * QT],
                                         lhsT=kT_b[rg, kc * 128:(kc + 1) * 128],
                                         rhs=qT_b[rg, qs])
                    e2 = ep.tile([128, 2 * QT], f16, name="expT")
                    nc.scalar.activation(e2[:], S2[:], Exp, scale=0.125)
                    for i in (0, 1):
                        kc = 2 * kc2 + i
                        nc.tensor.matmul(C2[:], lhsT=v_sb[kc][:, 2, :],
                                         rhs=e2[:, i * QT:(i + 1) * QT],
                                         start=(kc == 0), stop=(kc == KC - 1))
                normalize(C2, 2, qs)
                # output projection for these 512 query rows (4 seq tiles)
                for st in range(QT // 128):
                    r0 = qt * QT + st * 128
                    ws = slice(r0, r0 + 128)
                    po = ps_o.tile([128, D_MODEL], f32, name="po", tag="po")
                    for n, ns in enumerate((slice(0, 512), slice(512, 768))):
                        nc.tensor.matmul(po[:, ns], lhsT=ctxT_a[:, ws],
                                         rhs=wo_a[:, ns], start=True, stop=False)
                        nc.tensor.matmul(po[:, ns], lhsT=ctxT_b[:, ws],
                                         rhs=wo_b[:, ns], start=False, stop=True)
                    osb = op.tile([128, D_MODEL], f32, name="osb")
                    nc.vector.tensor_copy(out=osb[:], in_=po[:])
                    nc.sync.dma_start(out=out[ws, :], in_=osb[:])

    nc.compile()
    return nc


def _get_program():
    global _PROGRAM
    if _PROGRAM is None:
        _PROGRAM = _build_program()
    return _PROGRAM


def make_in_maps(query, key, value, Wq, bq, Wk, bk, Wv, bv, Wo, bo):
    """Build the 8 per-core input maps (host-side shard + transpose + cast)."""
    q32 = np.asarray(query, np.float32)
    k32 = np.asarray(key, np.float32)
    v32 = np.asarray(value, np.float32)
    xT = {}
    for b in range(B):
        xT[b] = (np.ascontiguousarray(q32[b].T).astype(np.float16),
                 np.ascontiguousarray(k32[b].T).astype(np.float16),
                 np.ascontiguousarray(v32[b].T).astype(np.float16))
    Wq = np.asarray(Wq, np.float32)
    Wk = np.asarray(Wk, np.float32)
    Wv = np.asarray(Wv, np.float32)
    Wo = np.asarray(Wo, np.float32)
    in_maps = []
    for c in range(N_CORES):
        b, g = divmod(c, G)
        fs = slice(g * GW, (g + 1) * GW)
        xq, xk, xv = xT[b]
        in_maps.append({
            "xqT": xq, "xkT": xk, "xvT": xv,
            "wq": np.ascontiguousarray(Wq[:, fs]).astype(np.float16),
            "wk": np.ascontiguousarray(Wk[:, fs]).astype(np.float16),
            "wv": np.ascontiguousarray(Wv[:, fs]).astype(np.float16),
            "wo": np.ascontiguousarray(Wo[fs, :]).astype(np.float16),
            "bq": np.asarray(bq, np.float32)[fs].reshape(GW, 1),
            "bk": np.asarray(bk, np.float32)[fs].reshape(GW, 1),
            "bv": np.asarray(bv, np.float32)[fs].reshape(GW, 1),
        })
    return in_maps


def combine_outputs(results, bo):
    """Sum the per-core partial outputs into the full [B, S, D] output."""
    bo = np.asarray(bo, np.float32)
    out = np.zeros((B, S, D_MODEL), np.float32)
    for c in range(N_CORES):
        b = c // G
        out[b] += np.asarray(results[c]["out"], np.float32)
    out += bo[None, None, :]
    return out


def kernel(**inputs):
    from concourse.bass_utils import run_bass_kernel_spmd

    nc = _get_program()
    in_maps = make_in_maps(**inputs)
    res = run_bass_kernel_spmd(nc, in_maps, list(range(N_CORES)))
    return combine_outputs(res.results, inputs["bo"])


# revision 19
# speedup vs baseline: 1.5319x; 1.2668x over previous
"""Multi-head attention (B=2, S=2048, D=768, H=12) on 8 trn2 NeuronCores.

Sharding: batch x head-group data/tensor parallel. Core c = b*4+g handles
batch b and heads [3g, 3g+3) (a 192-wide slice of the QKV projections and
the matching 192-row slice of Wo). Each core emits a partial [2048, 768]
output; the host sums the 4 head-group partials per batch and adds bo.

Device layout notes:
- Inputs are transposed on host to [d_model, seq] and cast to fp16 so the
  TensorEngine (which contracts over the partition dim) can consume them
  directly; all matmuls run on fp16 operands with fp32 PSUM accumulation.
- Attention works on transposed scores sT[k, q] so softmax's sum over k
  becomes a matmul reduction: v is augmented with a ones column, so the
  ctx matmul yields both ctx^T and the softmax denominator in one pass.
  exp() needs no max-subtraction: |scores/8| <= ~11 for this problem.
- Normalization multiplies ctx^T by 1/denom broadcast across partitions
  (GPSIMD partition_broadcast), then the output projection runs from
  ctx^T directly.
"""

import numpy as np

D_MODEL = 768
NUM_HEADS = 12
D_K = 64
B = 2
S = 2048
N_CORES = 8
G = 4            # head groups (cores per batch)
GW = D_MODEL // G  # 192 features per group = 3 heads
HPG = 3          # heads per group
DC = D_MODEL // 128  # 6 d_model chunks
QT = 512         # q-tile width
NQT = S // QT    # 4
KC = S // 128    # 16 k chunks
ST = S // 128    # 16 seq tiles

_PROGRAM = None


def _build_program():
    from concourse import bacc, tile
    import concourse.mybir as mybir

    f16 = mybir.dt.float16
    f32 = mybir.dt.float32
    Exp = mybir.ActivationFunctionType.Exp
    mult = mybir.AluOpType.mult

    nc = bacc.Bacc("TRN2", target_bir_lowering=False, debug=False,
                   enable_asserts=False)

    xqT = nc.dram_tensor("xqT", [D_MODEL, S], f16, kind="ExternalInput")
    xkT = nc.dram_tensor("xkT", [D_MODEL, S], f16, kind="ExternalInput")
    xvT = nc.dram_tensor("xvT", [D_MODEL, S], f16, kind="ExternalInput")
    wq = nc.dram_tensor("wq", [D_MODEL, GW], f16, kind="ExternalInput")
    wk = nc.dram_tensor("wk", [D_MODEL, GW], f16, kind="ExternalInput")
    wv = nc.dram_tensor("wv", [D_MODEL, GW], f16, kind="ExternalInput")
    wo = nc.dram_tensor("wo", [GW, D_MODEL], f16, kind="ExternalInput")
    bq = nc.dram_tensor("bq", [GW, 1], f32, kind="ExternalInput")
    bk = nc.dram_tensor("bk", [GW, 1], f32, kind="ExternalInput")
    bv = nc.dram_tensor("bv", [GW, 1], f32, kind="ExternalInput")
    out = nc.dram_tensor("out", [S, D_MODEL], f32, kind="ExternalOutput")

    with tile.TileContext(nc) as tc:
        with tc.tile_pool(name="const", bufs=1) as cp, \
             tc.tile_pool(name="expp", bufs=4) as ep, \
             tc.tile_pool(name="normp", bufs=2) as np_, \
             tc.tile_pool(name="outp", bufs=2) as op, \
             tc.tile_pool(name="ps_s", bufs=2, space="PSUM") as ps_s, \
             tc.tile_pool(name="ps_c", bufs=2, space="PSUM") as ps_c, \
             tc.tile_pool(name="ps_o", bufs=1, space="PSUM") as ps_o:

            wq_sb, wk_sb, wv_sb = [], [], []
            for nm, dram, lst in (("wq", wq, wq_sb), ("wk", wk, wk_sb),
                                  ("wv", wv, wv_sb)):
                for d in range(DC):
                    t = cp.tile([128, GW], f16, name=f"{nm}{d}")
                    nc.sync.dma_start(out=t[:], in_=dram[d * 128:(d + 1) * 128, :])
                    lst.append(t)
            wo_a = cp.tile([128, D_MODEL], f16, name="wo_a")
            nc.sync.dma_start(out=wo_a[:], in_=wo[0:128, :])
            wo_b = cp.tile([64, D_MODEL], f16, name="wo_b")
            nc.sync.dma_start(out=wo_b[:], in_=wo[128:GW, :])
            bq_a = cp.tile([128, 1], f32, name="bq_a")
            nc.sync.dma_start(out=bq_a[:], in_=bq[0:128, :])
            bq_b = cp.tile([64, 1], f32, name="bq_b")
            nc.sync.dma_start(out=bq_b[:], in_=bq[128:GW, :])
            bk_a = cp.tile([128, 1], f32, name="bk_a")
            nc.sync.dma_start(out=bk_a[:], in_=bk[0:128, :])
            bk_b = cp.tile([64, 1], f32, name="bk_b")
            nc.sync.dma_start(out=bk_b[:], in_=bk[128:GW, :])
            bv_h = []
            for h in range(HPG):
                t = cp.tile([64, 1], f32, name=f"bv{h}")
                nc.sync.dma_start(out=t[:], in_=bv[h * 64:(h + 1) * 64, :])
                bv_h.append(t)

            # ---- load inputs (k first, then v, then q: attention needs
            # full kT and v before it can start, q only per-tile) ----
            xq_sb = []
            xk_sb = []
            xv_sb = []
            for d in range(DC):
                t = cp.tile([128, S], f16, name=f"xk{d}")
                nc.sync.dma_start(out=t[:], in_=xkT[d * 128:(d + 1) * 128, :])
                xk_sb.append(t)
            for d in range(DC):
                t = cp.tile([128, S], f16, name=f"xv{d}")
                nc.sync.dma_start(out=t[:], in_=xvT[d * 128:(d + 1) * 128, :])
                xv_sb.append(t)
            for d in range(DC):
                t = cp.tile([128, S], f16, name=f"xq{d}")
                nc.sync.dma_start(out=t[:], in_=xqT[d * 128:(d + 1) * 128, :])
                xq_sb.append(t)
            # ---- projections. Order: kT, v, qT (attention dependency order)
            qT_a = cp.tile([128, S], f16, name="qT_a")
            qT_b = cp.tile([128, S], f16, name="qT_b")
            kT_a = cp.tile([128, S], f16, name="kT_a")
            kT_b = cp.tile([128, S], f16, name="kT_b")

            def qk_proj(x_sb, w_sb, b_a, b_b, dst_a, dst_b):
                # d-outer accumulation: each input chunk is consumed as it
                # arrives from HBM (no wait for the full tensor). Two PSUM
                # tiles [*, 1024] cover the whole row; the attention S-pool
                # is idle during projection, so borrow its slots.
                for half, (ms, b_t, dst, pp) in enumerate(
                        (((slice(0, 128)), b_a, dst_a, 128),
                         ((slice(128, GW)), b_b, dst_b, 64))):
                    pj = [ps_s.tile([pp, 2 * QT], f32, name="S", tag="s")
                          for _ in range(2)]
                    for d in range(DC):
                        for j2 in range(2):
                            for n in range(2):
                                cs = slice(j2 * 1024 + n * QT,
                                           j2 * 1024 + (n + 1) * QT)
                                nc.tensor.matmul(
                                    pj[j2][:, n * QT:(n + 1) * QT],
                                    lhsT=w_sb[d][:, ms], rhs=x_sb[d][:, cs],
                                    start=(d == 0), stop=(d == DC - 1))
                    for j2 in range(2):
                        js = slice(j2 * 1024, (j2 + 1) * 1024)
                        if half == 0:
                            nc.vector.tensor_scalar_add(dst[:, js],
                                                        pj[j2][:], b_t[:])
                        else:
                            nc.vector.tensor_scalar_add(dst[0:64, js],
                                                        pj[j2][:], b_t[:])
                # mirror the 64-row b-half into partitions 64-127 so head-2
                # score matmuls can alternate PE row groups (pairing)
                nc.sync.dma_start(out=dst_b[64:128, :], in_=dst_b[0:64, :])

            qk_proj(xk_sb, wk_sb, bk_a, bk_b, kT_a, kT_b)

            # v projection (natural layout) + ones column per head
            v_sb = []
            for st in range(ST):
                rs = slice(st * 128, (st + 1) * 128)
                pv = ps_c.tile([128, GW], f32, name="pj", tag="c")
                for d in range(DC):
                    nc.tensor.matmul(pv[:], lhsT=xv_sb[d][:, rs],
                                     rhs=wv_sb[d][:],
                                     start=(d == 0), stop=(d == DC - 1))
                vt = cp.tile([128, HPG, D_K + 1], f16, name=f"vsb{st}")
                nc.vector.tensor_copy(out=vt[:, :, 0:D_K],
                                      in_=pv.rearrange("p (h w) -> p h w", h=HPG))
                nc.vector.memset(vt[:, :, D_K:D_K + 1], 1.0)
                v_sb.append(vt)

            qk_proj(xq_sb, wq_sb, bq_a, bq_b, qT_a, qT_b)

            # ---- attention (transposed scores) + output projection ----
            ctxT_a = cp.tile([128, S], f16, name="ctxT_a")
            ctxT_b = cp.tile([64, S], f16, name="ctxT_b")

            def head_slices(h):
                if h == 0:
                    return kT_a[0:64], qT_a[0:64], ctxT_a[0:64]
                if h == 1:
                    return kT_a[64:128], qT_a[64:128], ctxT_a[64:128]
                return kT_b[0:64], qT_b[0:64], ctxT_b[0:64]

            def normalize(C, h, qs):
                # ctxT = C[0:64] * (1/denom) + bv.  reciprocal_approx_fast
                # must read SBUF (garbage from PSUM on HW), so stage the
                # denominator row through SBUF first.
                _, _, ctx_dst = head_slices(h)
                den = np_.tile([1, QT], f32, name="den")
                nc.vector.tensor_copy(out=den[:], in_=C[D_K:D_K + 1, :])
                r = np_.tile([1, QT], f32, name="r")
                nc.vector.reciprocal_approx_fast(out=r[:], in_=den[:])
                bc = np_.tile([128, QT], f32, name="bc")
                nc.gpsimd.partition_broadcast(bc[:], r[:])
                base = 64 if h == 1 else 0
                nc.vector.tensor_tensor(out=ctx_dst[:, qs],
                                        in0=C[0:D_K, :],
                                        in1=bc[base:base + D_K, :],
                                        op=mult)
                nc.vector.tensor_scalar_add(ctx_dst[:, qs], ctx_dst[:, qs],
                                            bv_h[h][:])

            for qt in range(NQT):
                qs = slice(qt * QT, (qt + 1) * QT)
                # heads 0+1 interleaved: scores for both go into one
                # [128, 1024] PSUM tile so exp runs as a single wide op,
                # and the two matmuls (row groups 0-63 / 64-127) overlap.
                Cs = {}
                for h in (0, 1):
                    Cs[h] = ps_c.tile([D_K + 1, QT], f32, name="C", tag="c")
                for kc in range(KC):
                    ks = slice(kc * 128, (kc + 1) * 128)
                    S2 = ps_s.tile([128, 2 * QT], f32, name="S", tag="s")
                    for h in (0, 1):
                        kT_h, qT_h, _ = head_slices(h)
                        nc.tensor.matmul(S2[:, h * QT:(h + 1) * QT],
                                         lhsT=kT_h[:, ks], rhs=qT_h[:, qs])
                    e2 = ep.tile([128, 2 * QT], f16, name="expT")
                    nc.scalar.activation(e2[:], S2[:], Exp, scale=0.125)
                    for h in (0, 1):
                        nc.tensor.matmul(Cs[h][:], lhsT=v_sb[kc][:, h, :],
                                         rhs=e2[:, h * QT:(h + 1) * QT],
                                         start=(kc == 0), stop=(kc == KC - 1))
                for h in (0, 1):
                    normalize(Cs[h], h, qs)
                # head 2: one [128, 1024] scores tile covers two k-chunks;
                # alternate PE row groups via the mirrored b-half
                C2 = ps_c.tile([D_K + 1, QT], f32, name="C", tag="c")
                for kc2 in range(KC // 2):
                    S2 = ps_s.tile([128, 2 * QT], f32, name="S", tag="s")
                    for i in (0, 1):
                        kc = 2 * kc2 + i
                        rg = slice(64 * i, 64 * i + 64)
                        nc.tensor.matmul(S2[:, i * QT:(i + 1) * QT],
                                         lhsT=kT_b[rg, kc * 128:(kc + 1) * 128],
                                         rhs=qT_b[rg, qs])
                    e2 = ep.tile([128, 2 * QT], f16, name="expT")
                    nc.scalar.activation(e2[:], S2[:], Exp, scale=0.125)
                    for i in (0, 1):
                        kc = 2 * kc2 + i
                        nc.tensor.matmul(C2[:], lhsT=v_sb[kc][:, 2, :],
                                         rhs=e2[:, i * QT:(i + 1) * QT],
                                         start=(kc == 0), stop=(kc == KC - 1))
                normalize(C2, 2, qs)
                # output projection for these 512 query rows (4 seq tiles)
                for st in range(QT // 128):
                    r0 = qt * QT + st * 128
                    ws = slice(r0, r0 + 128)
                    if qt == NQT - 1:
                        # attention is done: borrow the free S-pool slots so
                        # the tail output-projection pipelines
                        po = ps_s.tile([128, D_MODEL], f32, name="S", tag="s")
                    else:
                        po = ps_o.tile([128, D_MODEL], f32, name="po",
                                       tag="po")
                    for n, ns in enumerate((slice(0, 512), slice(512, 768))):
                        nc.tensor.matmul(po[:, ns], lhsT=ctxT_a[:, ws],
                                         rhs=wo_a[:, ns], start=True, stop=False)
                        nc.tensor.matmul(po[:, ns], lhsT=ctxT_b[:, ws],
                                         rhs=wo_b[:, ns], start=False, stop=True)
                    osb = op.tile([128, D_MODEL], f32, name="osb")
                    nc.vector.tensor_copy(out=osb[:], in_=po[:])
                    nc.sync.dma_start(out=out[ws, :], in_=osb[:])

    nc.compile()
    return nc


def _get_program():
    global _PROGRAM
    if _PROGRAM is None:
        _PROGRAM = _build_program()
    return _PROGRAM


def make_in_maps(query, key, value, Wq, bq, Wk, bk, Wv, bv, Wo, bo):
    """Build the 8 per-core input maps (host-side shard + transpose + cast)."""
    q32 = np.asarray(query, np.float32)
    k32 = np.asarray(key, np.float32)
    v32 = np.asarray(value, np.float32)
    xT = {}
    for b in range(B):
        xT[b] = (np.ascontiguousarray(q32[b].T).astype(np.float16),
                 np.ascontiguousarray(k32[b].T).astype(np.float16),
                 np.ascontiguousarray(v32[b].T).astype(np.float16))
    Wq = np.asarray(Wq, np.float32)
    Wk = np.asarray(Wk, np.float32)
    Wv = np.asarray(Wv, np.float32)
    Wo = np.asarray(Wo, np.float32)
    in_maps = []
    for c in range(N_CORES):
        b, g = divmod(c, G)
        fs = slice(g * GW, (g + 1) * GW)
        xq, xk, xv = xT[b]
        in_maps.append({
            "xqT": xq, "xkT": xk, "xvT": xv,
            "wq": np.ascontiguousarray(Wq[:, fs]).astype(np.float16),
            "wk": np.ascontiguousarray(Wk[:, fs]).astype(np.float16),
            "wv": np.ascontiguousarray(Wv[:, fs]).astype(np.float16),
            "wo": np.ascontiguousarray(Wo[fs, :]).astype(np.float16),
            "bq": np.asarray(bq, np.float32)[fs].reshape(GW, 1),
            "bk": np.asarray(bk, np.float32)[fs].reshape(GW, 1),
            "bv": np.asarray(bv, np.float32)[fs].reshape(GW, 1),
        })
    return in_maps


def combine_outputs(results, bo):
    """Sum the per-core partial outputs into the full [B, S, D] output."""
    bo = np.asarray(bo, np.float32)
    out = np.zeros((B, S, D_MODEL), np.float32)
    for c in range(N_CORES):
        b = c // G
        out[b] += np.asarray(results[c]["out"], np.float32)
    out += bo[None, None, :]
    return out


def kernel(**inputs):
    from concourse.bass_utils import run_bass_kernel_spmd

    nc = _get_program()
    in_maps = make_in_maps(**inputs)
    res = run_bass_kernel_spmd(nc, in_maps, list(range(N_CORES)))
    return combine_outputs(res.results, inputs["bo"])


# revision 20
# speedup vs baseline: 1.7432x; 1.1379x over previous
"""Multi-head attention (B=2, S=2048, D=768, H=12) on 8 trn2 NeuronCores.

Sharding: batch x head-group data/tensor parallel. Core c = b*4+g handles
batch b and heads [3g, 3g+3) (a 192-wide slice of the QKV projections and
the matching 192-row slice of Wo). Each core emits a partial [2048, 768]
output; the host sums the 4 head-group partials per batch and adds bo.

Device layout notes:
- Inputs are transposed on host to [d_model, seq] and cast to fp16 so the
  TensorEngine (which contracts over the partition dim) can consume them
  directly; all matmuls run on fp16 operands with fp32 PSUM accumulation.
- Attention works on transposed scores sT[k, q] so softmax's sum over k
  becomes a matmul reduction: v is augmented with a ones column, so the
  ctx matmul yields both ctx^T and the softmax denominator in one pass.
  exp() needs no max-subtraction: |scores/8| <= ~11 for this problem.
- Normalization multiplies ctx^T by 1/denom broadcast across partitions
  (GPSIMD partition_broadcast), then the output projection runs from
  ctx^T directly.
- Heads 0/1 live at SBUF partitions 0-63/64-127 so their score matmuls
  land in different PE row groups and overlap; head 2's operands are
  mirrored into both halves for the same reason.
- The output projection for q-tile j is emitted after q-tile j+1's
  attention so the PE stream never stalls on the normalize chain.
"""

import numpy as np

D_MODEL = 768
NUM_HEADS = 12
D_K = 64
B = 2
S = 2048
N_CORES = 8
G = 4              # head groups (cores per batch)
GW = D_MODEL // G  # 192 features per group = 3 heads
HPG = 3            # heads per group
DC = D_MODEL // 128  # 6 d_model chunks
QT = 512           # q-tile width
NQT = S // QT      # 4
KC = S // 128      # 16 k chunks
ST = S // 128      # 16 seq tiles
WPK = 3 * DC * GW + 2 * D_MODEL  # packed weights columns: 4992
BPK = 8            # packed bias columns

_PROGRAM = None


def _build_program():
    from concourse import bacc, tile
    import concourse.mybir as mybir

    f16 = mybir.dt.float16
    f32 = mybir.dt.float32
    Exp = mybir.ActivationFunctionType.Exp
    mult = mybir.AluOpType.mult

    nc = bacc.Bacc("TRN2", target_bir_lowering=False, debug=False,
                   enable_asserts=False)

    xqT = nc.dram_tensor("xqT", [D_MODEL, S], f16, kind="ExternalInput")
    xkT = nc.dram_tensor("xkT", [D_MODEL, S], f16, kind="ExternalInput")
    xvT = nc.dram_tensor("xvT", [D_MODEL, S], f16, kind="ExternalInput")
    wpk = nc.dram_tensor("wpk", [128, WPK], f16, kind="ExternalInput")
    bpk = nc.dram_tensor("bpk", [128, BPK], f32, kind="ExternalInput")
    out = nc.dram_tensor("out", [S, D_MODEL], f32, kind="ExternalOutput")

    with tile.TileContext(nc) as tc:
        with tc.tile_pool(name="const", bufs=1) as cp, \
             tc.tile_pool(name="expp", bufs=4) as ep, \
             tc.tile_pool(name="normp", bufs=2) as np_, \
             tc.tile_pool(name="outp", bufs=2) as op, \
             tc.tile_pool(name="ps_s", bufs=2, space="PSUM") as ps_s, \
             tc.tile_pool(name="ps_c", bufs=3, space="PSUM") as ps_c, \
             tc.tile_pool(name="ps_o", bufs=1, space="PSUM") as ps_o:

            # ---- packed weights + biases: two DMAs, then AP slices ----
            wps = cp.tile([128, WPK], f16, name="wps")
            nc.sync.dma_start(out=wps[:], in_=wpk[:, :])
            bps = cp.tile([128, BPK], f32, name="bps")
            nc.sync.dma_start(out=bps[:], in_=bpk[:, :])
            wq_sb = [wps[:, d * GW:(d + 1) * GW] for d in range(DC)]
            wk_sb = [wps[:, DC * GW + d * GW:DC * GW + (d + 1) * GW]
                     for d in range(DC)]
            wv_sb = [wps[:, 2 * DC * GW + d * GW:2 * DC * GW + (d + 1) * GW]
                     for d in range(DC)]
            wo_a = wps[:, 3 * DC * GW:3 * DC * GW + D_MODEL]
            wo_b = wps[0:64, 3 * DC * GW + D_MODEL:WPK]
            bq_a, bq_b = bps[:, 0:1], bps[0:64, 1:2]
            bk_a, bk_b = bps[:, 2:3], bps[0:64, 3:4]
            bv_h = [bps[0:64, 4 + h:5 + h] for h in range(HPG)]

            # ---- inputs: k first, then v, then q (attention needs full
            # kT and v before it can start, q only per-tile) ----
            xq_sb, xk_sb, xv_sb = [], [], []
            for d in range(DC):
                t = cp.tile([128, S], f16, name=f"xk{d}")
                nc.sync.dma_start(out=t[:], in_=xkT[d * 128:(d + 1) * 128, :])
                xk_sb.append(t)
            for d in range(DC):
                t = cp.tile([128, S], f16, name=f"xv{d}")
                nc.sync.dma_start(out=t[:], in_=xvT[d * 128:(d + 1) * 128, :])
                xv_sb.append(t)
            for d in range(DC):
                t = cp.tile([128, S], f16, name=f"xq{d}")
                nc.sync.dma_start(out=t[:], in_=xqT[d * 128:(d + 1) * 128, :])
                xq_sb.append(t)

            # ---- projections. Order: kT, v, qT (dependency order) ----
            qT_a = cp.tile([128, S], f16, name="qT_a")
            qT_b = cp.tile([128, S], f16, name="qT_b")
            kT_a = cp.tile([128, S], f16, name="kT_a")
            kT_b = cp.tile([128, S], f16, name="kT_b")

            def qk_proj(x_sb, w_sb, b_a, b_b, dst_a, dst_b):
                # d-outer accumulation: each input chunk is consumed as it
                # arrives from HBM. The attention S-pool is idle during
                # projection, so borrow its slots for the two row psums.
                for half, (ms, b_t, dst, pp) in enumerate(
                        ((slice(0, 128), b_a, dst_a, 128),
                         (slice(128, GW), b_b, dst_b, 64))):
                    pj = [ps_s.tile([pp, 2 * QT], f32, name="S", tag="s")
                          for _ in range(2)]
                    for d in range(DC):
                        for j2 in range(2):
                            for n in range(2):
                                cs = slice(j2 * 1024 + n * QT,
                                           j2 * 1024 + (n + 1) * QT)
                                nc.tensor.matmul(
                                    pj[j2][:, n * QT:(n + 1) * QT],
                                    lhsT=w_sb[d][:, ms], rhs=x_sb[d][:, cs],
                                    start=(d == 0), stop=(d == DC - 1))
                    for j2 in range(2):
                        js = slice(j2 * 1024, (j2 + 1) * 1024)
                        if half == 0:
                            nc.vector.tensor_scalar_add(dst[:, js],
                                                        pj[j2][:], b_t)
                        else:
                            nc.vector.tensor_scalar_add(dst[0:64, js],
                                                        pj[j2][:], b_t)
                # mirror the 64-row b-half into partitions 64-127 so head-2
                # score matmuls can alternate PE row groups (pairing)
                nc.sync.dma_start(out=dst_b[64:128, :], in_=dst_b[0:64, :])

            qk_proj(xk_sb, wk_sb, bk_a, bk_b, kT_a, kT_b)

            # v projection (natural layout) + ones column per head
            v_sb = []
            for st in range(ST):
                rs = slice(st * 128, (st + 1) * 128)
                pv = ps_c.tile([128, GW], f32, name="pj", tag="c")
                for d in range(DC):
                    nc.tensor.matmul(pv[:], lhsT=xv_sb[d][:, rs],
                                     rhs=wv_sb[d][:],
                                     start=(d == 0), stop=(d == DC - 1))
                vt = cp.tile([128, HPG, D_K + 1], f16, name=f"vsb{st}")
                nc.vector.tensor_copy(out=vt[:, :, 0:D_K],
                                      in_=pv.rearrange("p (h w) -> p h w",
                                                       h=HPG))
                nc.vector.memset(vt[:, :, D_K:D_K + 1], 1.0)
                v_sb.append(vt)

            qk_proj(xq_sb, wq_sb, bq_a, bq_b, qT_a, qT_b)

            # ---- attention (transposed scores) + output projection ----
            ctxT_a = cp.tile([128, S], f16, name="ctxT_a")
            ctxT_b = cp.tile([64, S], f16, name="ctxT_b")

            def head_slices(h):
                if h == 0:
                    return kT_a[0:64], qT_a[0:64], ctxT_a[0:64]
                if h == 1:
                    return kT_a[64:128], qT_a[64:128], ctxT_a[64:128]
                return kT_b[0:64], qT_b[0:64], ctxT_b[0:64]

            def normalize(C, h, qs):
                # ctxT = C[0:64] * (1/denom) + bv.  reciprocal_approx_fast
                # must read SBUF (garbage from PSUM on HW), so stage the
                # denominator row through SBUF first.
                _, _, ctx_dst = head_slices(h)
                den = np_.tile([1, QT], f32, name="den")
                nc.vector.tensor_copy(out=den[:], in_=C[D_K:D_K + 1, :])
                r = np_.tile([1, QT], f32, name="r")
                nc.vector.reciprocal_approx_fast(out=r[:], in_=den[:])
                bc = np_.tile([128, QT], f32, name="bc")
                nc.gpsimd.partition_broadcast(bc[:], r[:])
                base = 64 if h == 1 else 0
                nc.vector.tensor_tensor(out=ctx_dst[:, qs],
                                        in0=C[0:D_K, :],
                                        in1=bc[base:base + D_K, :],
                                        op=mult)
                nc.vector.tensor_scalar_add(ctx_dst[:, qs], ctx_dst[:, qs],
                                            bv_h[h])

            def attention(qt):
                qs = slice(qt * QT, (qt + 1) * QT)
                # heads 0+1 interleaved: both go into one [128, 1024] PSUM
                # tile so exp runs as a single wide op, and the two score
                # matmuls (row groups 0-63 / 64-127) overlap on the PE.
                Cs = {}
                for h in (0, 1):
                    Cs[h] = ps_c.tile([D_K + 1, QT], f32, name="C", tag="c")
                for kc in range(KC):
                    ks = slice(kc * 128, (kc + 1) * 128)
                    S2 = ps_s.tile([128, 2 * QT], f32, name="S", tag="s")
                    for h in (0, 1):
                        kT_h, qT_h, _ = head_slices(h)
                        nc.tensor.matmul(S2[:, h * QT:(h + 1) * QT],
                                         lhsT=kT_h[:, ks], rhs=qT_h[:, qs])
                    e2 = ep.tile([128, 2 * QT], f16, name="expT")
                    nc.scalar.activation(e2[:], S2[:], Exp, scale=0.125)
                    for h in (0, 1):
                        nc.tensor.matmul(Cs[h][:], lhsT=v_sb[kc][:, h, :],
                                         rhs=e2[:, h * QT:(h + 1) * QT],
                                         start=(kc == 0), stop=(kc == KC - 1))
                for h in (0, 1):
                    normalize(Cs[h], h, qs)
                # head 2: one [128, 1024] scores tile covers two k-chunks;
                # alternate PE row groups via the mirrored b-half
                C2 = ps_c.tile([D_K + 1, QT], f32, name="C", tag="c")
                for kc2 in range(KC // 2):
                    S2 = ps_s.tile([128, 2 * QT], f32, name="S", tag="s")
                    for i in (0, 1):
                        kc = 2 * kc2 + i
                        rg = slice(64 * i, 64 * i + 64)
                        nc.tensor.matmul(
                            S2[:, i * QT:(i + 1) * QT],
                            lhsT=kT_b[rg, kc * 128:(kc + 1) * 128],
                            rhs=qT_b[rg, qs])
                    e2 = ep.tile([128, 2 * QT], f16, name="expT")
                    nc.scalar.activation(e2[:], S2[:], Exp, scale=0.125)
                    for i in (0, 1):
                        kc = 2 * kc2 + i
                        nc.tensor.matmul(C2[:], lhsT=v_sb[kc][:, 2, :],
                                         rhs=e2[:, i * QT:(i + 1) * QT],
                                         start=(kc == 0), stop=(kc == KC - 1))
                normalize(C2, 2, qs)

            def out_proj(qt, last=False):
                for st in range(QT // 128):
                    r0 = qt * QT + st * 128
                    ws = slice(r0, r0 + 128)
                    osb = op.tile([128, D_MODEL], f32, name="osb")
                    for n, ns in enumerate((slice(0, 384), slice(384, 768))):
                        if last:
                            # attention is done: borrow the free S-pool
                            # slots so the tail pipelines
                            po = ps_s.tile([128, 384], f32, name="S",
                                           tag="s")
                        else:
                            po = ps_o.tile([128, 384], f32, name="po",
                                           tag="po")
                        nc.tensor.matmul(po[:], lhsT=ctxT_a[:, ws],
                                         rhs=wo_a[:, ns],
                                         start=True, stop=False)
                        nc.tensor.matmul(po[:], lhsT=ctxT_b[:, ws],
                                         rhs=wo_b[:, ns],
                                         start=False, stop=True)
                        nc.vector.tensor_copy(out=osb[:, ns], in_=po[:])
                    nc.sync.dma_start(out=out[ws, :], in_=osb[:])

            # software pipeline: out_proj(qt) is emitted after
            # attention(qt+1) so the PE never waits on the normalize chain
            for qt in range(NQT):
                attention(qt)
                if qt > 0:
                    out_proj(qt - 1)
            out_proj(NQT - 1, last=True)

    nc.compile()
    return nc


def _get_program():
    global _PROGRAM
    if _PROGRAM is None:
        _PROGRAM = _build_program()
    return _PROGRAM


def make_in_maps(query, key, value, Wq, bq, Wk, bk, Wv, bv, Wo, bo):
    """Build the 8 per-core input maps (host-side shard + transpose + cast)."""
    q32 = np.asarray(query, np.float32)
    k32 = np.asarray(key, np.float32)
    v32 = np.asarray(value, np.float32)
    xT = {}
    for b in range(B):
        xT[b] = (np.ascontiguousarray(q32[b].T).astype(np.float16),
                 np.ascontiguousarray(k32[b].T).astype(np.float16),
                 np.ascontiguousarray(v32[b].T).astype(np.float16))
    Wq = np.asarray(Wq, np.float32)
    Wk = np.asarray(Wk, np.float32)
    Wv = np.asarray(Wv, np.float32)
    Wo = np.asarray(Wo, np.float32)
    bq = np.asarray(bq, np.float32)
    bk = np.asarray(bk, np.float32)
    bv = np.asarray(bv, np.float32)
    in_maps = []
    for c in range(N_CORES):
        b, g = divmod(c, G)
        fs = slice(g * GW, (g + 1) * GW)
        xq, xk, xv = xT[b]
        # packed weights [128, WPK]: wq|wk|wv chunks (d-major), wo_a, wo_b
        wps = np.zeros((128, WPK), np.float16)
        for i, W in enumerate((Wq, Wk, Wv)):
            Ws = W[:, fs]
            for d in range(DC):
                wps[:, (i * DC + d) * GW:(i * DC + d + 1) * GW] = \
                    Ws[d * 128:(d + 1) * 128, :].astype(np.float16)
        Wos = Wo[fs, :]
        wps[:, 3 * DC * GW:3 * DC * GW + D_MODEL] = \
            Wos[0:128, :].astype(np.float16)
        wps[0:64, 3 * DC * GW + D_MODEL:WPK] = \
            Wos[128:GW, :].astype(np.float16)
        # packed biases [128, 8] f32
        bps = np.zeros((128, BPK), np.float32)
        bps[:, 0] = bq[fs][0:128]
        bps[0:64, 1] = bq[fs][128:GW]
        bps[:, 2] = bk[fs][0:128]
        bps[0:64, 3] = bk[fs][128:GW]
        for h in range(HPG):
            bps[0:64, 4 + h] = bv[fs][h * 64:(h + 1) * 64]
        in_maps.append({
            "xqT": xq, "xkT": xk, "xvT": xv,
            "wpk": wps, "bpk": bps,
        })
    return in_maps


def combine_outputs(results, bo):
    """Sum the per-core partial outputs into the full [B, S, D] output."""
    bo = np.asarray(bo, np.float32)
    out = np.zeros((B, S, D_MODEL), np.float32)
    for c in range(N_CORES):
        b = c // G
        out[b] += np.asarray(results[c]["out"], np.float32)
    out += bo[None, None, :]
    return out


def kernel(**inputs):
    from concourse.bass_utils import run_bass_kernel_spmd

    nc = _get_program()
    in_maps = make_in_maps(**inputs)
    res = run_bass_kernel_spmd(nc, in_maps, list(range(N_CORES)))
    return combine_outputs(res.results, inputs["bo"])


# revision 23
# speedup vs baseline: 1.8095x; 1.0380x over previous
"""Multi-head attention (B=2, S=2048, D=768, H=12) on 8 trn2 NeuronCores.

Sharding: batch x head-group data/tensor parallel. Core c = b*4+g handles
batch b and heads [3g, 3g+3) (a 192-wide slice of the QKV projections and
the matching 192-row slice of Wo). Each core emits a partial [2048, 768]
output; the host sums the 4 head-group partials per batch and adds bo.

Device layout notes:
- Inputs are transposed on host to [d_model, seq] and cast to fp16 so the
  TensorEngine (which contracts over the partition dim) can consume them
  directly; all matmuls run on fp16 operands with fp32 PSUM accumulation.
- Attention works on transposed scores sT[k, q] so softmax's sum over k
  becomes a matmul reduction: v is augmented with a ones column, so the
  ctx matmul yields both ctx^T and the softmax denominator in one pass.
  exp() needs no max-subtraction: |scores/8| <= ~11 for this problem.
- Normalization multiplies ctx^T by 1/denom broadcast across partitions
  (GPSIMD partition_broadcast), then the output projection runs from
  ctx^T directly.
- Heads 0/1 live at SBUF partitions 0-63/64-127 so their score matmuls
  land in different PE row groups and overlap; head 2's operands are
  mirrored into both halves for the same reason.
- The output projection for q-tile j is emitted after q-tile j+1's
  attention so the PE stream never stalls on the normalize chain.
"""

import numpy as np

D_MODEL = 768
NUM_HEADS = 12
D_K = 64
B = 2
S = 2048
N_CORES = 8
G = 4              # head groups (cores per batch)
GW = D_MODEL // G  # 192 features per group = 3 heads
HPG = 3            # heads per group
DC = D_MODEL // 128  # 6 d_model chunks
QT = 512           # q-tile width
NQT = S // QT      # 4
KC = S // 128      # 16 k chunks
ST = S // 128      # 16 seq tiles
WPK = 3 * DC * GW + 2 * D_MODEL  # packed weights columns: 4992
BPK = 8            # packed bias columns

_PROGRAM = None


def _build_program():
    from concourse import bacc, tile
    import concourse.mybir as mybir

    f16 = mybir.dt.float16
    f32 = mybir.dt.float32
    Exp = mybir.ActivationFunctionType.Exp
    mult = mybir.AluOpType.mult

    nc = bacc.Bacc("TRN2", target_bir_lowering=False, debug=False,
                   enable_asserts=False)

    xqT = nc.dram_tensor("xqT", [D_MODEL, S], f16, kind="ExternalInput")
    xkT = nc.dram_tensor("xkT", [D_MODEL, S], f16, kind="ExternalInput")
    xvT = nc.dram_tensor("xvT", [D_MODEL, S], f16, kind="ExternalInput")
    wpk = nc.dram_tensor("wpk", [128, WPK], f16, kind="ExternalInput")
    bpk = nc.dram_tensor("bpk", [128, BPK], f32, kind="ExternalInput")
    out = nc.dram_tensor("out", [S, D_MODEL], f32, kind="ExternalOutput")

    with tile.TileContext(nc) as tc:
        with tc.tile_pool(name="const", bufs=1) as cp, \
             tc.tile_pool(name="expp", bufs=6) as ep, \
             tc.tile_pool(name="normp", bufs=2) as np_, \
             tc.tile_pool(name="outp", bufs=2) as op, \
             tc.tile_pool(name="ps_s", bufs=2, space="PSUM") as ps_s, \
             tc.tile_pool(name="ps_c", bufs=3, space="PSUM") as ps_c, \
             tc.tile_pool(name="ps_o", bufs=1, space="PSUM") as ps_o:

            # ---- packed weights + biases: two DMAs, then AP slices ----
            wps = cp.tile([128, WPK], f16, name="wps")
            nc.sync.dma_start(out=wps[:], in_=wpk[:, :])
            bps = cp.tile([128, BPK], f32, name="bps")
            nc.sync.dma_start(out=bps[:], in_=bpk[:, :])
            wq_sb = [wps[:, d * GW:(d + 1) * GW] for d in range(DC)]
            wk_sb = [wps[:, DC * GW + d * GW:DC * GW + (d + 1) * GW]
                     for d in range(DC)]
            wv_sb = [wps[:, 2 * DC * GW + d * GW:2 * DC * GW + (d + 1) * GW]
                     for d in range(DC)]
            wo_a = wps[:, 3 * DC * GW:3 * DC * GW + D_MODEL]
            wo_b = wps[0:64, 3 * DC * GW + D_MODEL:WPK]
            bq_a, bq_b = bps[:, 0:1], bps[0:64, 1:2]
            bk_a, bk_b = bps[:, 2:3], bps[0:64, 3:4]
            bv_h = [bps[0:64, 4 + h:5 + h] for h in range(HPG)]

            # ---- inputs: k first, then v, then q (attention needs full
            # kT and v before it can start, q only per-tile) ----
            xq_sb, xk_sb, xv_sb = [], [], []
            for d in range(DC):
                t = cp.tile([128, S], f16, name=f"xk{d}")
                nc.sync.dma_start(out=t[:], in_=xkT[d * 128:(d + 1) * 128, :])
                xk_sb.append(t)
            for d in range(DC):
                t = cp.tile([128, S], f16, name=f"xq{d}")
                nc.sync.dma_start(out=t[:], in_=xqT[d * 128:(d + 1) * 128, :])
                xq_sb.append(t)
            for d in range(DC):
                t = cp.tile([128, S], f16, name=f"xv{d}")
                nc.sync.dma_start(out=t[:], in_=xvT[d * 128:(d + 1) * 128, :])
                xv_sb.append(t)

            # ---- projections. Order: kT, v, qT (dependency order) ----
            qT_a = cp.tile([128, S], f16, name="qT_a")
            qT_b = cp.tile([128, S], f16, name="qT_b")
            kT_a = cp.tile([128, S], f16, name="kT_a")
            kT_b = cp.tile([128, S], f16, name="kT_b")

            def qk_proj(x_sb, w_sb, b_a, b_b, dst_a, dst_b):
                # d-outer accumulation: each input chunk is consumed as it
                # arrives from HBM. The attention S-pool is idle during
                # projection, so borrow its slots for the two row psums.
                for half, (ms, b_t, dst, pp) in enumerate(
                        ((slice(0, 128), b_a, dst_a, 128),
                         (slice(128, GW), b_b, dst_b, 64))):
                    pj = [ps_s.tile([pp, 2 * QT], f32, name="S", tag="s")
                          for _ in range(2)]
                    for d in range(DC):
                        for j2 in range(2):
                            for n in range(2):
                                cs = slice(j2 * 1024 + n * QT,
                                           j2 * 1024 + (n + 1) * QT)
                                nc.tensor.matmul(
                                    pj[j2][:, n * QT:(n + 1) * QT],
                                    lhsT=w_sb[d][:, ms], rhs=x_sb[d][:, cs],
                                    start=(d == 0), stop=(d == DC - 1))
                    for j2 in range(2):
                        js = slice(j2 * 1024, (j2 + 1) * 1024)
                        if half == 0:
                            nc.vector.tensor_scalar_add(dst[:, js],
                                                        pj[j2][:], b_t)
                        else:
                            nc.vector.tensor_scalar_add(dst[0:64, js],
                                                        pj[j2][:], b_t)
                # mirror the 64-row b-half into partitions 64-127 so head-2
                # score matmuls can alternate PE row groups (pairing)
                nc.sync.dma_start(out=dst_b[64:128, :], in_=dst_b[0:64, :])

            qk_proj(xk_sb, wk_sb, bk_a, bk_b, kT_a, kT_b)
            qk_proj(xq_sb, wq_sb, bq_a, bq_b, qT_a, qT_b)

            # v projection (natural layout) + ones column per head.
            # Emitted per seq-tile, fused into q-tile 0's attention loop so
            # the ACT exp stream starts before v finishes projecting.
            v_sb = [None] * ST

            def v_proj(st):
                rs = slice(st * 128, (st + 1) * 128)
                pv = ps_c.tile([128, GW], f32, name="pj", tag="c")
                for d in range(DC):
                    nc.tensor.matmul(pv[:], lhsT=xv_sb[d][:, rs],
                                     rhs=wv_sb[d][:],
                                     start=(d == 0), stop=(d == DC - 1))
                vt = cp.tile([128, HPG, D_K + 1], f16, name=f"vsb{st}")
                nc.vector.tensor_copy(out=vt[:, :, 0:D_K],
                                      in_=pv.rearrange("p (h w) -> p h w",
                                                       h=HPG))
                nc.vector.memset(vt[:, :, D_K:D_K + 1], 1.0)
                v_sb[st] = vt

            # ---- attention (transposed scores) + output projection ----
            ctxT_a = cp.tile([128, S], f16, name="ctxT_a")
            ctxT_b = cp.tile([64, S], f16, name="ctxT_b")

            def head_slices(h):
                if h == 0:
                    return kT_a[0:64], qT_a[0:64], ctxT_a[0:64]
                if h == 1:
                    return kT_a[64:128], qT_a[64:128], ctxT_a[64:128]
                return kT_b[0:64], qT_b[0:64], ctxT_b[0:64]

            def normalize(C, h, qs):
                # ctxT = C[0:64] * (1/denom) + bv.  reciprocal_approx_fast
                # must read SBUF (garbage from PSUM on HW), so stage the
                # denominator row through SBUF first.
                _, _, ctx_dst = head_slices(h)
                den = np_.tile([1, QT], f32, name="den")
                nc.vector.tensor_copy(out=den[:], in_=C[D_K:D_K + 1, :])
                r = np_.tile([1, QT], f32, name="r")
                nc.vector.reciprocal_approx_fast(out=r[:], in_=den[:])
                bc = np_.tile([128, QT], f32, name="bc")
                nc.gpsimd.partition_broadcast(bc[:], r[:])
                base = 64 if h == 1 else 0
                nc.vector.tensor_tensor(out=ctx_dst[:, qs],
                                        in0=C[0:D_K, :],
                                        in1=bc[base:base + D_K, :],
                                        op=mult)
                nc.vector.tensor_scalar_add(ctx_dst[:, qs], ctx_dst[:, qs],
                                            bv_h[h])

            def attn_hp01(qt, fuse_v):
                # heads 0+1 interleaved: both go into one [128, 1024] PSUM
                # tile so exp runs as a single wide op, and the two score
                # matmuls (row groups 0-63 / 64-127) overlap on the PE.
                qs = slice(qt * QT, (qt + 1) * QT)
                Cs = {}
                for h in (0, 1):
                    Cs[h] = ps_c.tile([D_K + 1, QT], f32, name="C", tag="c")
                for kc in range(KC):
                    ks = slice(kc * 128, (kc + 1) * 128)
                    S2 = ps_s.tile([128, 2 * QT], f32, name="S", tag="s")
                    for h in (0, 1):
                        kT_h, qT_h, _ = head_slices(h)
                        nc.tensor.matmul(S2[:, h * QT:(h + 1) * QT],
                                         lhsT=kT_h[:, ks], rhs=qT_h[:, qs])
                    e2 = ep.tile([128, 2 * QT], f16, name="expT")
                    nc.scalar.activation(e2[:], S2[:], Exp, scale=0.125)
                    if fuse_v:
                        v_proj(kc)
                    for h in (0, 1):
                        nc.tensor.matmul(Cs[h][:], lhsT=v_sb[kc][:, h, :],
                                         rhs=e2[:, h * QT:(h + 1) * QT],
                                         start=(kc == 0), stop=(kc == KC - 1))
                for h in (0, 1):
                    normalize(Cs[h], h, qs)

            def attn_h2(qt):
                # head 2: one [128, 1024] scores tile covers two k-chunks;
                # alternate PE row groups via the mirrored b-half
                qs = slice(qt * QT, (qt + 1) * QT)
                C2 = ps_c.tile([D_K + 1, QT], f32, name="C", tag="c")
                for kc2 in range(KC // 2):
                    S2 = ps_s.tile([128, 2 * QT], f32, name="S", tag="s")
                    for i in (0, 1):
                        kc = 2 * kc2 + i
                        rg = slice(64 * i, 64 * i + 64)
                        nc.tensor.matmul(
                            S2[:, i * QT:(i + 1) * QT],
                            lhsT=kT_b[rg, kc * 128:(kc + 1) * 128],
                            rhs=qT_b[rg, qs])
                    e2 = ep.tile([128, 2 * QT], f16, name="expT")
                    nc.scalar.activation(e2[:], S2[:], Exp, scale=0.125)
                    for i in (0, 1):
                        kc = 2 * kc2 + i
                        nc.tensor.matmul(C2[:], lhsT=v_sb[kc][:, 2, :],
                                         rhs=e2[:, i * QT:(i + 1) * QT],
                                         start=(kc == 0), stop=(kc == KC - 1))
                normalize(C2, 2, qs)

            def attention(qt):
                if qt == NQT - 1:
                    # last q-tile: head 2 first, so the tail's normalize +
                    # output projection waits on the shorter h0/h1 chain
                    attn_h2(qt)
                    attn_hp01(qt, fuse_v=False)
                else:
                    attn_hp01(qt, fuse_v=(qt == 0))
                    attn_h2(qt)

            def out_proj(qt, last=False):
                for st in range(QT // 128):
                    r0 = qt * QT + st * 128
                    ws = slice(r0, r0 + 128)
                    osb = op.tile([128, D_MODEL], f32, name="osb")
                    for n, ns in enumerate((slice(0, 384), slice(384, 768))):
                        if last:
                            # attention is done: borrow the free S-pool
                            # slots so the tail pipelines
                            po = ps_s.tile([128, 384], f32, name="S",
                                           tag="s")
                        else:
                            po = ps_o.tile([128, 384], f32, name="po",
                                           tag="po")
                        nc.tensor.matmul(po[:], lhsT=ctxT_a[:, ws],
                                         rhs=wo_a[:, ns],
                                         start=True, stop=False)
                        nc.tensor.matmul(po[:], lhsT=ctxT_b[:, ws],
                                         rhs=wo_b[:, ns],
                                         start=False, stop=True)
                        nc.vector.tensor_copy(out=osb[:, ns], in_=po[:])
                    nc.sync.dma_start(out=out[ws, :], in_=osb[:])

            # software pipeline: out_proj(qt) is emitted after
            # attention(qt+1) so the PE never waits on the normalize chain
            for qt in range(NQT):
                attention(qt)
                if qt > 0:
                    out_proj(qt - 1)
            out_proj(NQT - 1, last=True)

    nc.compile()
    return nc


def _get_program():
    global _PROGRAM
    if _PROGRAM is None:
        _PROGRAM = _build_program()
    return _PROGRAM


def make_in_maps(query, key, value, Wq, bq, Wk, bk, Wv, bv, Wo, bo):
    """Build the 8 per-core input maps (host-side shard + transpose + cast)."""
    q32 = np.asarray(query, np.float32)
    k32 = np.asarray(key, np.float32)
    v32 = np.asarray(value, np.float32)
    xT = {}
    for b in range(B):
        xT[b] = (np.ascontiguousarray(q32[b].T).astype(np.float16),
                 np.ascontiguousarray(k32[b].T).astype(np.float16),
                 np.ascontiguousarray(v32[b].T).astype(np.float16))
    Wq = np.asarray(Wq, np.float32)
    Wk = np.asarray(Wk, np.float32)
    Wv = np.asarray(Wv, np.float32)
    Wo = np.asarray(Wo, np.float32)
    bq = np.asarray(bq, np.float32)
    bk = np.asarray(bk, np.float32)
    bv = np.asarray(bv, np.float32)
    in_maps = []
    for c in range(N_CORES):
        b, g = divmod(c, G)
        fs = slice(g * GW, (g + 1) * GW)
        xq, xk, xv = xT[b]
        # packed weights [128, WPK]: wq|wk|wv chunks (d-major), wo_a, wo_b
        wps = np.zeros((128, WPK), np.float16)
        for i, W in enumerate((Wq, Wk, Wv)):
            Ws = W[:, fs]
            for d in range(DC):
                wps[:, (i * DC + d) * GW:(i * DC + d + 1) * GW] = \
                    Ws[d * 128:(d + 1) * 128, :].astype(np.float16)
        Wos = Wo[fs, :]
        wps[:, 3 * DC * GW:3 * DC * GW + D_MODEL] = \
            Wos[0:128, :].astype(np.float16)
        wps[0:64, 3 * DC * GW + D_MODEL:WPK] = \
            Wos[128:GW, :].astype(np.float16)
        # packed biases [128, 8] f32
        bps = np.zeros((128, BPK), np.float32)
        bps[:, 0] = bq[fs][0:128]
        bps[0:64, 1] = bq[fs][128:GW]
        bps[:, 2] = bk[fs][0:128]
        bps[0:64, 3] = bk[fs][128:GW]
        for h in range(HPG):
            bps[0:64, 4 + h] = bv[fs][h * 64:(h + 1) * 64]
        in_maps.append({
            "xqT": xq, "xkT": xk, "xvT": xv,
            "wpk": wps, "bpk": bps,
        })
    return in_maps


def combine_outputs(results, bo):
    """Sum the per-core partial outputs into the full [B, S, D] output."""
    bo = np.asarray(bo, np.float32)
    out = np.zeros((B, S, D_MODEL), np.float32)
    for c in range(N_CORES):
        b = c // G
        out[b] += np.asarray(results[c]["out"], np.float32)
    out += bo[None, None, :]
    return out


def kernel(**inputs):
    from concourse.bass_utils import run_bass_kernel_spmd

    nc = _get_program()
    in_maps = make_in_maps(**inputs)
    res = run_bass_kernel_spmd(nc, in_maps, list(range(N_CORES)))
    return combine_outputs(res.results, inputs["bo"])
